# revision 12
# baseline (speedup 1.0000x reference)
"""Trainium2 Bass kernel for full-fisheye warping (bilinear grid sample).

Strategy
--------
The warp grid (source coords + bilinear weights) depends only on (K, dist)
and the output pixel position - it is shared across all batch/channel
planes.  We shard the OUTPUT ROWS across the 8 NeuronCores (48 rows each);
each core processes all 96 (batch, channel) planes for its rows.

Host side (cheap numpy, done once per call):
  * compute the warp grid exactly like the reference (float32 ops)
  * build, per core, a "banded" bf16 copy of the image with vertically
    interleaved row pairs: item (y, x) = (I[y,x], I[y+1,x]) packed in 4
    bytes.  One gathered f32-typed element therefore yields BOTH vertical
    taps of one column; two gathers per pixel yield the full 2x2 patch.
  * build int16 gather-index tables and interleaved bf16 weight tables
Device side (one SPMD NEFF on 8 cores):
  * DMA band/idx/weight tiles in, GPSIMD ap_gather 2 items per output
    pixel (indices shared across the 16 planes of each partition group),
    DVE blend (elementwise mul + pair reductions, all bf16), DMA out.
Host side: descramble the [step, partition, px] output into (B, C, OH, OW).
"""

import math
import numpy as np
import ml_dtypes

BF16 = ml_dtypes.bfloat16

# ---------------------------------------------------------------- constants
B, C, H, W = 32, 3, 960, 1280
CROP = 0.3
CH, CW = int(H * CROP), int(W * CROP)          # 288, 384
OH, OW = H - 2 * CH, W - 2 * CW                # 384, 512
NDEV = 8
ROWS_PER_DEV = OH // NDEV                      # 48
NPLANE = B * C                                 # 96
NPB = NPLANE // 16                             # 6 plane blocks of 16
UR, UC = 24, 64                                # unit (tile) = 24 rows x 64 cols
NRC = ROWS_PER_DEV // UR                       # 2 row-chunks
NCC = OW // UC                                 # 8 col-chunks
STEPS = NPB * NRC                              # 12 schedule steps
NPX = UR * UC                                  # 1536 px per partition per step
NIDX = 2 * NPX                                 # 3072 gather items per group

_cache: dict = {}


# ------------------------------------------------------------- host compute
def _warp_grid(K: np.ndarray, dist: np.ndarray):
    """Replicate the reference grid computation in float32."""
    f32 = np.float32
    K = K.astype(np.float32)
    dist = dist.astype(np.float32)
    y, x = np.meshgrid(np.arange(H, dtype=np.float32),
                       np.arange(W, dtype=np.float32), indexing="ij")
    xn = (x - K[0, 2]) / K[0, 0]
    yn = (y - K[1, 2]) / K[1, 1]
    r2 = xn * xn + yn * yn
    k1, k2, k3, k4 = dist[0], dist[1], dist[2], dist[3]
    dfac = f32(1.0) + r2 * (k1 + r2 * (k2 + r2 * (k3 + r2 * k4)))
    xd = xn * dfac * K[0, 0] + K[0, 2]
    yd = yn * dfac * K[1, 1] + K[1, 2]
    xd = xd[CH:H - CH, CW:W - CW]
    yd = yd[CH:H - CH, CW:W - CW]
    ix0f = np.floor(xd)
    iy0f = np.floor(yd)
    wx1 = xd - ix0f
    wy1 = yd - iy0f
    wx0 = f32(1.0) - wx1
    wy0 = f32(1.0) - wy1
    ix0 = ix0f.astype(np.int32)
    iy0 = iy0f.astype(np.int32)

    # zeros-padding validity (reference semantics); folded into the weights
    def val(iy, ix):
        return ((iy >= 0) & (iy < H) & (ix >= 0) & (ix < W)).astype(np.float32)

    w00 = (wy0 * wx0) * val(iy0, ix0)
    w01 = (wy0 * wx1) * val(iy0, ix0 + 1)
    w10 = (wy1 * wx0) * val(iy0 + 1, ix0)
    w11 = (wy1 * wx1) * val(iy0 + 1, ix0 + 1)
    iy0c = np.clip(iy0, 0, H - 1)
    ix0c = np.clip(ix0, 0, W - 1)
    iy1c = np.clip(iy0 + 1, 0, H - 1)
    ix1c = np.clip(ix0 + 1, 0, W - 1)
    return iy0c, ix0c, iy1c, ix1c, w00, w01, w10, w11


def _build_tables(K, dist):
    """Geometry tables shared by graph build + data staging."""
    iy0, ix0, iy1, ix1, w00, w01, w10, w11 = _warp_grid(K, dist)

    # Band rects per (rc, c): uniform bh/bw across devices (required: one
    # SPMD NEFF), per-device by0/bx0.
    by0 = np.zeros((NDEV, NRC, NCC), np.int32)
    bx0 = np.zeros((NDEV, NRC, NCC), np.int32)
    bh = np.zeros((NRC, NCC), np.int32)
    bw = np.zeros((NRC, NCC), np.int32)
    for rc in range(NRC):
        for c in range(NCC):
            for d in range(NDEV):
                r0 = d * ROWS_PER_DEV + rc * UR
                sy0 = iy0[r0:r0 + UR, c * UC:(c + 1) * UC]
                sy1 = iy1[r0:r0 + UR, c * UC:(c + 1) * UC]
                sx0 = ix0[r0:r0 + UR, c * UC:(c + 1) * UC]
                sx1 = ix1[r0:r0 + UR, c * UC:(c + 1) * UC]
                by0[d, rc, c] = sy0.min()
                bx0[d, rc, c] = sx0.min()
                bh[rc, c] = max(bh[rc, c], sy1.max() + 1 - sy0.min())
                bw[rc, c] = max(bw[rc, c], sx1.max() + 1 - sx0.min())
    # clamp rects inside the image (pad-rows from uniformization)
    for d in range(NDEV):
        by0[d] = np.minimum(by0[d], H - bh)
        bx0[d] = np.minimum(bx0[d], W - bw)

    bhv = bh - 1                        # vertical-pair rows in the band
    emax = int((bhv * bw).max())        # f32 items per partition band
    offs = np.zeros((STEPS, NCC), np.int64)
    total = 0
    for s in range(STEPS):
        rc = s % NRC
        for g in range(NCC):
            offs[s, g] = total
            total += 16 * int(bhv[rc, g] * bw[rc, g])

    # per-device idx + weight tables
    idx_t = np.zeros((NDEV, STEPS, 128, NIDX // 16), np.int16)
    w_t = np.zeros((NDEV, STEPS, 128, 4 * NPX), BF16)
    for d in range(NDEV):
        for s in range(STEPS):
            rc = s % NRC
            for g in range(NCC):
                r0 = d * ROWS_PER_DEV + rc * UR
                cs = slice(g * UC, (g + 1) * UC)
                rs = slice(r0, r0 + UR)
                bwv = int(bw[rc, g])
                ly0 = iy0[rs, cs] - by0[d, rc, g]
                lx0 = ix0[rs, cs] - bx0[d, rc, g]
                lx1 = ix1[rs, cs] - bx0[d, rc, g]
                iA = (ly0 * bwv + lx0).ravel()        # item -> (t00, t10)
                iB = (ly0 * bwv + lx1).ravel()        # item -> (t01, t11)
                # interleave: per px [iA, iB]
                lst = np.stack([iA, iB], axis=1).reshape(-1).astype(np.int16)
                assert lst.min() >= 0 and lst.max() < bhv[rc, g] * bwv
                idx_t[d, s, 16 * g:16 * (g + 1), :] = lst.reshape(-1, 16).T
                # weights interleaved to match bf16 view of gathered items:
                # per px [w00, w10, w01, w11]
                wq = np.stack([w00[rs, cs].ravel(), w10[rs, cs].ravel(),
                               w01[rs, cs].ravel(), w11[rs, cs].ravel()],
                              axis=1).reshape(-1).astype(BF16)
                w_t[d, s, 16 * g:16 * (g + 1), :] = wq[None, :]

    return dict(by0=by0, bx0=bx0, bh=bh, bw=bw, bhv=bhv, emax=emax, offs=offs,
                total=total, idx_t=idx_t, w_t=w_t)


def _stage_bands(img, tab):
    """Per-device flat vertical-pair band buffers (bf16 pairs in f32 view)."""
    planes = img.reshape(NPLANE, H, W).astype(BF16)
    bh, bw, by0, bx0, offs = tab["bh"], tab["bw"], tab["by0"], tab["bx0"], tab["offs"]
    bands = np.empty((NDEV, tab["total"] * 2), BF16)
    for d in range(NDEV):
        buf = bands[d]
        for s in range(STEPS):
            pb, rc = s // NRC, s % NRC
            for g in range(NCC):
                hh, ww = int(bh[rc, g]), int(bw[rc, g])
                yy, xx = int(by0[d, rc, g]), int(bx0[d, rc, g])
                patch = planes[16 * pb:16 * (pb + 1), yy:yy + hh, xx:xx + ww]
                # vertical pair interleave: item (y, x) = (I[y,x], I[y+1,x])
                pv = np.stack([patch[:, :-1, :], patch[:, 1:, :]], axis=-1)
                o = int(offs[s, g]) * 2
                buf[o:o + pv.size] = pv.reshape(-1)
    return bands.view(np.float32)


# ------------------------------------------------------------- device graph
def _build_graph(tab, repeats=1, mode="full"):
    import concourse.bass as bass
    import concourse.tile as tile
    from concourse import bacc, mybir
    from contextlib import ExitStack

    f32 = mybir.dt.float32
    bf16 = mybir.dt.bfloat16
    i16 = mybir.dt.int16
    emax = tab["emax"]
    bhv, bw, offs = tab["bhv"], tab["bw"], tab["offs"]

    nc = bacc.Bacc("TRN2", target_bir_lowering=False, debug=False, num_devices=NDEV)
    bands_p = nc.dram_tensor("bands", [tab["total"]], f32, kind="ExternalInput")
    idx_p = nc.dram_tensor("idx", [STEPS, 128, NIDX // 16], i16, kind="ExternalInput")
    w_p = nc.dram_tensor("w", [STEPS, 128, 4 * NPX], bf16, kind="ExternalInput")
    out_p = nc.dram_tensor("out", [STEPS, 128, NPX], bf16, kind="ExternalOutput")

    with tile.TileContext(nc) as tc, ExitStack() as ctx:
        bandp = ctx.enter_context(tc.tile_pool(name="band", bufs=3))
        idxp = ctx.enter_context(tc.tile_pool(name="idxp", bufs=2))
        wp = ctx.enter_context(tc.tile_pool(name="wp", bufs=2))
        gp = ctx.enter_context(tc.tile_pool(name="gp", bufs=2))
        tp = ctx.enter_context(tc.tile_pool(name="tp", bufs=2))
        op = ctx.enter_context(tc.tile_pool(name="op", bufs=2))

        for s in range(STEPS * repeats):
            s = s % STEPS
            rc = s % NRC
            band_t = bandp.tile([128, emax], f32)
            if mode != "gatheronly":
                for g in range(NCC):
                    e = int(bhv[rc, g] * bw[rc, g])
                    o = int(offs[s, g])
                    src = bands_p[o:o + 16 * e].rearrange("(p e) -> p e", p=16)
                    nc.sync.dma_start(band_t[16 * g:16 * (g + 1), 0:e], src)
            else:
                nc.sync.dma_start(band_t[:, 0:16],
                                  bands_p[0:128 * 16].rearrange("(p e) -> p e", p=128))
            idx_t = idxp.tile([128, NIDX // 16], i16)
            nc.scalar.dma_start(idx_t[:], idx_p[s, :, :])
            w_t = wp.tile([128, 4 * NPX], bf16)
            nc.scalar.dma_start(w_t[:], w_p[s, :, :])

            if mode == "dma":
                o_t = op.tile([128, NPX], bf16)
                nc.vector.tensor_copy(o_t[:], w_t[:, 0:NPX])
                nc.scalar.dma_start(out_p[s, :, :], o_t[:])
                continue

            gat = gp.tile([128, NIDX], f32)
            nc.gpsimd.ap_gather(gat[:], band_t[:], idx_t[:],
                                channels=128, num_elems=emax, d=1, num_idxs=NIDX)

            if mode in ("gather", "gatheronly"):
                nc.scalar.dma_start(out_p[s, :, :],
                                  gat[:, 0:NPX].bitcast(bf16)[:, 0:NPX])
                continue

            # blend: P = G * W elementwise (bf16, dense, 2x mode), then
            # pair-reductions: out = (P0+P2) + (P1+P3) per px quad
            gb = gat[:].bitcast(bf16)                 # [128, 4*NPX]
            p_t = tp.tile([128, 4 * NPX], bf16, tag="p")
            nc.vector.tensor_mul(p_t[:], gb, w_t[:])
            p3 = p_t[:].rearrange("p (n k) -> p n k", k=4)
            s_t = tp.tile([128, 2 * NPX], bf16, tag="s2")
            s3 = s_t[:].rearrange("p (n k) -> p n k", k=2)
            nc.vector.tensor_add(s3, p3[:, :, 0:2], p3[:, :, 2:4])
            o_t = op.tile([128, NPX], bf16)
            s2 = s_t[:].rearrange("p (n k) -> p n k", k=2)
            nc.vector.tensor_add(o_t[:].rearrange("p (n k) -> p n k", k=1),
                                 s2[:, :, 0:1], s2[:, :, 1:2])

            nc.scalar.dma_start(out_p[s, :, :], o_t[:])

    nc.compile()
    return nc


def _descramble(outs):
    """[NDEV][STEPS,128,NPX] bf16 -> (B, C, OH, OW) f32."""
    full = np.empty((NPLANE, OH, OW), np.float32)
    for d in range(NDEV):
        o = np.asarray(outs[d]).astype(np.float32).reshape(
            NPB, NRC, NCC, 16, UR, UC)
        # dims: (pb, rc, g, j, r, cl) -> (pb, j, rc, r, g, cl)
        blk = o.transpose(0, 3, 1, 4, 2, 5).reshape(NPLANE, ROWS_PER_DEV, OW)
        full[:, d * ROWS_PER_DEV:(d + 1) * ROWS_PER_DEV, :] = blk
    return full.reshape(B, C, OH, OW)


# ---------------------------------------------------------------- interface
def _get_built(K, dist):
    key = (np.asarray(K, np.float32).tobytes(), np.asarray(dist, np.float32).tobytes())
    if key not in _cache:
        tab = _build_tables(K, dist)
        nc = _build_graph(tab)
        _cache[key] = (tab, nc)
    return _cache[key]


def kernel(img: np.ndarray, K: np.ndarray, dist: np.ndarray) -> np.ndarray:
    from concourse.bass_utils import run_bass_kernel_spmd

    img = np.asarray(img, np.float32)
    assert img.shape == (B, C, H, W), img.shape
    tab, nc = _get_built(np.asarray(K), np.asarray(dist))
    bands = _stage_bands(img, tab)
    in_maps = [
        {"bands": bands[d], "idx": tab["idx_t"][d], "w": tab["w_t"][d]}
        for d in range(NDEV)
    ]
    res = run_bass_kernel_spmd(nc, in_maps, core_ids=list(range(NDEV)))
    outs = [res.results[d]["out"] for d in range(NDEV)]
    return _descramble(outs)


# revision 14
# speedup vs baseline: 1.1128x; 1.1128x over previous
"""Trainium2 Bass kernel for full-fisheye warping (bilinear grid sample).

Strategy
--------
The warp grid (source coords + bilinear weights) depends only on (K, dist)
and the output pixel position - it is shared across all batch/channel
planes.  We shard the OUTPUT ROWS across the 8 NeuronCores (48 rows each);
each core processes all 96 (batch, channel) planes for its rows.

Host side (cheap numpy, done once per call):
  * compute the warp grid exactly like the reference (float32 ops)
  * build, per core, a "banded" bf16 copy of the image with vertically
    interleaved row pairs: item (y, x) = (I[y,x], I[y+1,x]) packed in 4
    bytes.  One gathered f32-typed element therefore yields BOTH vertical
    taps of one column; two gathers per pixel yield the full 2x2 patch.
  * build int16 gather-index tables and interleaved bf16 weight tables
Device side (one SPMD NEFF on 8 cores):
  * DMA band/idx/weight tiles in, GPSIMD ap_gather 2 items per output
    pixel (indices shared across the 16 planes of each partition group),
    DVE blend (elementwise mul + pair reductions, all bf16), DMA out.
Host side: descramble the [step, partition, px] output into (B, C, OH, OW).
"""

import math
import numpy as np
import ml_dtypes

BF16 = ml_dtypes.bfloat16

# ---------------------------------------------------------------- constants
B, C, H, W = 32, 3, 960, 1280
CROP = 0.3
CH, CW = int(H * CROP), int(W * CROP)          # 288, 384
OH, OW = H - 2 * CH, W - 2 * CW                # 384, 512
NDEV = 8
ROWS_PER_DEV = OH // NDEV                      # 48
NPLANE = B * C                                 # 96
NPB = NPLANE // 16                             # 6 plane blocks of 16
UR, UC = 24, 64                                # unit (tile) = 24 rows x 64 cols
NRC = ROWS_PER_DEV // UR                       # 2 row-chunks
NCC = OW // UC                                 # 8 col-chunks
STEPS = NPB * NRC                              # 12 schedule steps
NPX = UR * UC                                  # 1536 px per partition per step
NIDX = 2 * NPX                                 # 3072 gather items per group

_cache: dict = {}


# ------------------------------------------------------------- host compute
def _warp_grid(K: np.ndarray, dist: np.ndarray):
    """Replicate the reference grid computation in float32."""
    f32 = np.float32
    K = K.astype(np.float32)
    dist = dist.astype(np.float32)
    y, x = np.meshgrid(np.arange(H, dtype=np.float32),
                       np.arange(W, dtype=np.float32), indexing="ij")
    xn = (x - K[0, 2]) / K[0, 0]
    yn = (y - K[1, 2]) / K[1, 1]
    r2 = xn * xn + yn * yn
    k1, k2, k3, k4 = dist[0], dist[1], dist[2], dist[3]
    dfac = f32(1.0) + r2 * (k1 + r2 * (k2 + r2 * (k3 + r2 * k4)))
    xd = xn * dfac * K[0, 0] + K[0, 2]
    yd = yn * dfac * K[1, 1] + K[1, 2]
    xd = xd[CH:H - CH, CW:W - CW]
    yd = yd[CH:H - CH, CW:W - CW]
    ix0f = np.floor(xd)
    iy0f = np.floor(yd)
    wx1 = xd - ix0f
    wy1 = yd - iy0f
    wx0 = f32(1.0) - wx1
    wy0 = f32(1.0) - wy1
    ix0 = ix0f.astype(np.int32)
    iy0 = iy0f.astype(np.int32)

    # zeros-padding validity (reference semantics); folded into the weights
    def val(iy, ix):
        return ((iy >= 0) & (iy < H) & (ix >= 0) & (ix < W)).astype(np.float32)

    w00 = (wy0 * wx0) * val(iy0, ix0)
    w01 = (wy0 * wx1) * val(iy0, ix0 + 1)
    w10 = (wy1 * wx0) * val(iy0 + 1, ix0)
    w11 = (wy1 * wx1) * val(iy0 + 1, ix0 + 1)
    iy0c = np.clip(iy0, 0, H - 1)
    ix0c = np.clip(ix0, 0, W - 1)
    iy1c = np.clip(iy0 + 1, 0, H - 1)
    ix1c = np.clip(ix0 + 1, 0, W - 1)
    return iy0c, ix0c, iy1c, ix1c, w00, w01, w10, w11


def _build_tables(K, dist):
    """Geometry tables shared by graph build + data staging."""
    iy0, ix0, iy1, ix1, w00, w01, w10, w11 = _warp_grid(K, dist)

    # Band rects per (rc, c): uniform bh/bw across devices (required: one
    # SPMD NEFF), per-device by0/bx0.
    by0 = np.zeros((NDEV, NRC, NCC), np.int32)
    bx0 = np.zeros((NDEV, NRC, NCC), np.int32)
    bh = np.zeros((NRC, NCC), np.int32)
    bw = np.zeros((NRC, NCC), np.int32)
    for rc in range(NRC):
        for c in range(NCC):
            for d in range(NDEV):
                r0 = d * ROWS_PER_DEV + rc * UR
                sy0 = iy0[r0:r0 + UR, c * UC:(c + 1) * UC]
                sy1 = iy1[r0:r0 + UR, c * UC:(c + 1) * UC]
                sx0 = ix0[r0:r0 + UR, c * UC:(c + 1) * UC]
                sx1 = ix1[r0:r0 + UR, c * UC:(c + 1) * UC]
                by0[d, rc, c] = sy0.min()
                bx0[d, rc, c] = sx0.min()
                bh[rc, c] = max(bh[rc, c], sy1.max() + 1 - sy0.min())
                bw[rc, c] = max(bw[rc, c], sx1.max() + 1 - sx0.min())
    # clamp rects inside the image (pad-rows from uniformization)
    for d in range(NDEV):
        by0[d] = np.minimum(by0[d], H - bh)
        bx0[d] = np.minimum(bx0[d], W - bw)

    bhv = bh - 1                        # vertical-pair rows in the band
    emax = int((bhv * bw).max())        # f32 items per partition band
    offs = np.zeros((STEPS, NCC), np.int64)
    total = 0
    for s in range(STEPS):
        rc = s % NRC
        for g in range(NCC):
            offs[s, g] = total
            total += 16 * int(bhv[rc, g] * bw[rc, g])

    # per-device idx + weight tables
    idx_t = np.zeros((NDEV, STEPS, 128, NIDX // 16), np.int16)
    w_t = np.zeros((NDEV, STEPS, 128, 4 * NPX), BF16)
    for d in range(NDEV):
        for s in range(STEPS):
            rc = s % NRC
            for g in range(NCC):
                r0 = d * ROWS_PER_DEV + rc * UR
                cs = slice(g * UC, (g + 1) * UC)
                rs = slice(r0, r0 + UR)
                bwv = int(bw[rc, g])
                ly0 = iy0[rs, cs] - by0[d, rc, g]
                lx0 = ix0[rs, cs] - bx0[d, rc, g]
                lx1 = ix1[rs, cs] - bx0[d, rc, g]
                iA = (ly0 * bwv + lx0).ravel()        # item -> (t00, t10)
                iB = (ly0 * bwv + lx1).ravel()        # item -> (t01, t11)
                # interleave: per px [iA, iB]
                lst = np.stack([iA, iB], axis=1).reshape(-1).astype(np.int16)
                assert lst.min() >= 0 and lst.max() < bhv[rc, g] * bwv
                idx_t[d, s, 16 * g:16 * (g + 1), :] = lst.reshape(-1, 16).T
                # weights interleaved to match bf16 view of gathered items:
                # per px [w00, w10, w01, w11]
                wq = np.stack([w00[rs, cs].ravel(), w10[rs, cs].ravel(),
                               w01[rs, cs].ravel(), w11[rs, cs].ravel()],
                              axis=1).reshape(-1).astype(BF16)
                w_t[d, s, 16 * g:16 * (g + 1), :] = wq[None, :]

    return dict(by0=by0, bx0=bx0, bh=bh, bw=bw, bhv=bhv, emax=emax, offs=offs,
                total=total, idx_t=idx_t, w_t=w_t)


def _stage_bands(img, tab):
    """Per-device flat vertical-pair band buffers (bf16 pairs in f32 view)."""
    planes = img.reshape(NPLANE, H, W).astype(BF16)
    bh, bw, by0, bx0, offs = tab["bh"], tab["bw"], tab["by0"], tab["bx0"], tab["offs"]
    bands = np.empty((NDEV, tab["total"] * 2), BF16)
    for d in range(NDEV):
        buf = bands[d]
        for s in range(STEPS):
            pb, rc = s // NRC, s % NRC
            for g in range(NCC):
                hh, ww = int(bh[rc, g]), int(bw[rc, g])
                yy, xx = int(by0[d, rc, g]), int(bx0[d, rc, g])
                patch = planes[16 * pb:16 * (pb + 1), yy:yy + hh, xx:xx + ww]
                # vertical pair interleave: item (y, x) = (I[y,x], I[y+1,x])
                pv = np.stack([patch[:, :-1, :], patch[:, 1:, :]], axis=-1)
                o = int(offs[s, g]) * 2
                buf[o:o + pv.size] = pv.reshape(-1)
    return bands.view(np.float32)


# ------------------------------------------------------------- device graph
def _build_graph(tab, repeats=1, mode="full"):
    import concourse.bass as bass
    import concourse.tile as tile
    from concourse import bacc, mybir
    from contextlib import ExitStack

    f32 = mybir.dt.float32
    bf16 = mybir.dt.bfloat16
    i16 = mybir.dt.int16
    emax = tab["emax"]
    bhv, bw, offs = tab["bhv"], tab["bw"], tab["offs"]

    nc = bacc.Bacc("TRN2", target_bir_lowering=False, debug=False, num_devices=NDEV)
    bands_p = nc.dram_tensor("bands", [tab["total"]], f32, kind="ExternalInput")
    idx_p = nc.dram_tensor("idx", [STEPS, 128, NIDX // 16], i16, kind="ExternalInput")
    w_p = nc.dram_tensor("w", [STEPS, 128, 4 * NPX], bf16, kind="ExternalInput")
    out_p = nc.dram_tensor("out", [STEPS, 128, NPX], bf16, kind="ExternalOutput")

    with tile.TileContext(nc) as tc, ExitStack() as ctx:
        bandp = ctx.enter_context(tc.tile_pool(name="band", bufs=3))
        idxp = ctx.enter_context(tc.tile_pool(name="idxp", bufs=2))
        wp = ctx.enter_context(tc.tile_pool(name="wp", bufs=2))
        gp = ctx.enter_context(tc.tile_pool(name="gp", bufs=2))
        tp = ctx.enter_context(tc.tile_pool(name="tp", bufs=2))
        op = ctx.enter_context(tc.tile_pool(name="op", bufs=2))

        for s in range(STEPS * repeats):
            s = s % STEPS
            rc = s % NRC
            band_t = bandp.tile([128, emax], f32)
            if mode == "gatheronly":
                nc.sync.dma_start(band_t[:, 0:16],
                                  bands_p[0:128 * 16].rearrange("(p e) -> p e", p=128))
            else:
                shrink = 4 if mode == "gather_qb" else 1
                for g in range(NCC):
                    e = int(bhv[rc, g] * bw[rc, g]) // shrink
                    o = int(offs[s, g])
                    src = bands_p[o:o + 16 * e].rearrange("(p e) -> p e", p=16)
                    nc.sync.dma_start(band_t[16 * g:16 * (g + 1), 0:e], src)
            idx_t = idxp.tile([128, NIDX // 16], i16)
            nc.scalar.dma_start(idx_t[:], idx_p[s, :, :])
            if mode != "gather_now":
                w_t = wp.tile([128, 4 * NPX], bf16)
                nc.scalar.dma_start(w_t[:], w_p[s, :, :])

            if mode == "dma":
                o_t = op.tile([128, NPX], bf16)
                nc.vector.tensor_copy(o_t[:], w_t[:, 0:NPX])
                nc.scalar.dma_start(out_p[s, :, :], o_t[:])
                continue

            gat = gp.tile([128, NIDX], f32)
            nc.gpsimd.ap_gather(gat[:], band_t[:], idx_t[:],
                                channels=128, num_elems=emax, d=1, num_idxs=NIDX)

            if mode in ("gather", "gatheronly", "gather_now", "gather_qb"):
                nc.scalar.dma_start(out_p[s, :, :],
                                  gat[:, 0:NPX].bitcast(bf16)[:, 0:NPX])
                continue

            # blend: P = G * W elementwise (bf16, dense, 2x mode), then
            # pair-reductions: out = (P0+P2) + (P1+P3) per px quad
            gb = gat[:].bitcast(bf16)                 # [128, 4*NPX]
            p_t = tp.tile([128, 4 * NPX], bf16, tag="p")
            nc.vector.tensor_mul(p_t[:], gb, w_t[:])
            p3 = p_t[:].rearrange("p (n k) -> p n k", k=4)
            s_t = tp.tile([128, 2 * NPX], bf16, tag="s2")
            s3 = s_t[:].rearrange("p (n k) -> p n k", k=2)
            nc.vector.tensor_add(s3, p3[:, :, 0:2], p3[:, :, 2:4])
            o_t = op.tile([128, NPX], bf16)
            s2 = s_t[:].rearrange("p (n k) -> p n k", k=2)
            nc.vector.tensor_add(o_t[:].rearrange("p (n k) -> p n k", k=1),
                                 s2[:, :, 0:1], s2[:, :, 1:2])

            nc.scalar.dma_start(out_p[s, :, :], o_t[:])

    nc.compile()
    return nc


def _descramble(outs):
    """[NDEV][STEPS,128,NPX] bf16 -> (B, C, OH, OW) f32."""
    full = np.empty((NPLANE, OH, OW), np.float32)
    for d in range(NDEV):
        o = np.asarray(outs[d]).astype(np.float32).reshape(
            NPB, NRC, NCC, 16, UR, UC)
        # dims: (pb, rc, g, j, r, cl) -> (pb, j, rc, r, g, cl)
        blk = o.transpose(0, 3, 1, 4, 2, 5).reshape(NPLANE, ROWS_PER_DEV, OW)
        full[:, d * ROWS_PER_DEV:(d + 1) * ROWS_PER_DEV, :] = blk
    return full.reshape(B, C, OH, OW)


# ---------------------------------------------------------------- interface
def _get_built(K, dist):
    key = (np.asarray(K, np.float32).tobytes(), np.asarray(dist, np.float32).tobytes())
    if key not in _cache:
        tab = _build_tables(K, dist)
        nc = _build_graph(tab)
        _cache[key] = (tab, nc)
    return _cache[key]


def kernel(img: np.ndarray, K: np.ndarray, dist: np.ndarray) -> np.ndarray:
    from concourse.bass_utils import run_bass_kernel_spmd

    img = np.asarray(img, np.float32)
    assert img.shape == (B, C, H, W), img.shape
    tab, nc = _get_built(np.asarray(K), np.asarray(dist))
    bands = _stage_bands(img, tab)
    in_maps = [
        {"bands": bands[d], "idx": tab["idx_t"][d], "w": tab["w_t"][d]}
        for d in range(NDEV)
    ]
    res = run_bass_kernel_spmd(nc, in_maps, core_ids=list(range(NDEV)))
    outs = [res.results[d]["out"] for d in range(NDEV)]
    return _descramble(outs)
